# revision 2
# baseline (speedup 1.0000x reference)
"""Trainium2 Bass kernel for nn_RelGraphEncoderTG (2-layer RGCN encoder).

Self-contained: takes the FULL inputs (as produced by the problem's
setup_inputs), shards across 8 NeuronCores internally, and returns the FULL
[65536, 256] float32 output.

Algorithm / sharding
--------------------
Nodes are split contiguously across the 8 cores (8192 per core); edges are
partitioned by destination-node owner.  Per core, edges are bucketed by
destination into windows of 8 consecutive nodes; each window owns one
128-edge-slot "chunk" (padded; pads carry zero weight).  The per-relation
mean-normalised, basis-decomposed message aggregation is refactored as

    agg[v] = sum_b S[v,b,:] @ basis[b]     with
    S[v,b,:] = sum_{e: dst(e)=v} w[e,b] * h[src(e)],
    w[e,b]   = comp[type(e),b] / cnt[dst(e), type(e)]

so the sparse scatter becomes dense PE matmuls: per chunk

    S^T[dhalf, (b,slot)] += Xg[128e, 128d].T @ M[128e, 64]

where Xg holds gathered source features (layer 1: pre-gathered on the host
from the input x; layer 2: per-chunk indirect DMA from the all-gathered h1)
and M is a host-built weighted one-hot (w[e,b] at column b*8+slot).  Per
group of 128 nodes, agg accumulates 16 basis matmuls + 2 root matmuls in
PSUM, followed by exact gelu (ACT), residual and LayerNorm (fused DVE ops).
h1 is all-gathered in fp16 between the layers.  All matmul operands are
fp16 with fp32 PSUM accumulation.
"""
import numpy as np

import concourse.bass as bass
import concourse.mybir as mybir
import concourse.tile as tile
import concourse.bacc as bacc
from concourse.bass_utils import run_bass_kernel_spmd

F16 = mybir.dt.float16
F32 = mybir.dt.float32
I32 = mybir.dt.int32
AF = mybir.ActivationFunctionType
ALU = mybir.AluOpType

NCORES = 8
NTOT = 65536
SHARD = NTOT // NCORES       # 8192 nodes per core
D = 256
NB = 8
R = 32
L = 2
WIN = 8                      # dst nodes per window (= per chunk)
CPG = 16                     # chunks per group (group = 128 dst nodes)
NW = SHARD // WIN            # windows == chunks per core per layer (1024)
G = SHARD // 128             # node groups per core (64)
LN_EPS = 1e-5


def _host_prep(inputs):
    x = np.asarray(inputs["x_flat"], np.float32)
    ei = np.asarray(inputs["edge_index"], np.int64)
    et = np.asarray(inputs["edge_type"], np.int64).astype(np.int32)
    basis = np.asarray(inputs["basis"], np.float32)
    comp = np.asarray(inputs["comp"], np.float32)
    root = np.asarray(inputs["root"], np.float32)
    bias = np.asarray(inputs["bias"], np.float32)
    gamma = np.asarray(inputs["ln_gamma"], np.float32)
    beta = np.asarray(inputs["ln_beta"], np.float32)
    mask = np.asarray(inputs["valid_mask_flat"])

    assert mask.all(), "kernel assumes valid_mask all-true (spec fill=ones)"
    has_bias = bool(np.any(bias != 0))
    has_affine = bool(np.any(gamma != 1) or np.any(beta != 0))

    src = ei[0].astype(np.int64)
    dst = ei[1].astype(np.int64)
    seg = dst * R + et
    cnt = np.bincount(seg, minlength=NTOT * R).astype(np.float32)
    norm = 1.0 / np.maximum(cnt[seg], 1.0)
    w = comp[:, et, :] * norm[None, :, None]        # [L, E, NB]

    x16 = x.astype(np.float16)

    per_core = []
    for k in range(NCORES):
        lo, hi = k * SHARD, (k + 1) * SHARD
        eidx = np.nonzero((dst >= lo) & (dst < hi))[0]
        dk = dst[eidx] - lo
        win = (dk >> 3).astype(np.int64)
        slot = (dk & 7).astype(np.int64)
        order = np.argsort(win, kind="stable")
        eidx, win, slot = eidx[order], win[order], slot[order]
        counts = np.bincount(win, minlength=NW)
        assert counts.max() <= 128, f"window degree {counts.max()} > 128"
        starts = np.concatenate([[0], np.cumsum(counts)[:-1]])
        pos = np.arange(len(eidx)) - starts[win]

        idx_t = np.zeros((128, NW), np.int32)       # pad -> row 0 (zero weight)
        idx_t[pos, win] = src[eidx].astype(np.int32)
        xg1 = np.zeros((128, NW, D), np.float16)
        xg1[pos, win, :] = x16[src[eidx]]
        m_tab = np.zeros((L, 128, NW, NB * WIN), np.float16)
        cols = (np.arange(NB)[None, :] * WIN) + slot[:, None]
        for l in range(L):
            m_tab[l, pos[:, None], win[:, None], cols] = \
                w[l, eidx, :].astype(np.float16)

        per_core.append({
            "x_full": x16,
            "x_shard": x16[lo:hi],
            "idx_t": idx_t,
            "xg1": xg1,
            "m_tab": m_tab,
            "basis_in": np.ascontiguousarray(
                basis.reshape(L, NB, 2, 128, D).astype(np.float16)),
            "root_in": np.ascontiguousarray(
                root.reshape(L, 2, 128, D).astype(np.float16)),
            "bias_in": np.ascontiguousarray(
                np.broadcast_to(bias[:, None, :], (L, 128, D)).astype(np.float16)),
            "ln_aff": np.ascontiguousarray(
                np.stack([np.broadcast_to(gamma[:, None, :], (L, 128, D)),
                          np.broadcast_to(beta[:, None, :], (L, 128, D))],
                         axis=1).astype(np.float32)),
        })
    return per_core, has_bias, has_affine


def build_nc(has_bias=False, has_affine=False, reps=1):
    nc = bacc.Bacc("TRN2", target_bir_lowering=False, debug=False,
                   enable_asserts=False, num_devices=NCORES)

    x_full = nc.dram_tensor("x_full", [NTOT, D], F16, kind="ExternalInput")
    x_shard = nc.dram_tensor("x_shard", [SHARD, D], F16, kind="ExternalInput")
    idx_in = nc.dram_tensor("idx_t", [128, NW], I32, kind="ExternalInput")
    xg1_in = nc.dram_tensor("xg1", [128, NW, D], F16, kind="ExternalInput")
    m_in = nc.dram_tensor("m_tab", [L, 128, NW, NB * WIN], F16,
                          kind="ExternalInput")
    basis_in = nc.dram_tensor("basis_in", [L, NB, 2, 128, D], F16,
                              kind="ExternalInput")
    root_in = nc.dram_tensor("root_in", [L, 2, 128, D], F16,
                             kind="ExternalInput")
    bias_in = nc.dram_tensor("bias_in", [L, 128, D], F16, kind="ExternalInput")
    aff_in = nc.dram_tensor("ln_aff", [L, 2, 128, D], F32, kind="ExternalInput")
    out_t = nc.dram_tensor("out", [SHARD, D], F16, kind="ExternalOutput")

    rg = [list(range(NCORES))]

    with tile.TileContext(nc) as tc:
        with tc.tile_pool(name="const", bufs=1) as constp, \
             tc.tile_pool(name="gin", bufs=4) as ginp, \
             tc.tile_pool(name="mst", bufs=4) as mstp, \
             tc.tile_pool(name="sT", bufs=3) as sTp, \
             tc.tile_pool(name="ep", bufs=3) as epp, \
             tc.tile_pool(name="sc_ps", bufs=6, space="PSUM") as scps, \
             tc.tile_pool(name="agg_ps", bufs=2, space="PSUM") as aggps, \
             tc.tile_pool(name="dram", bufs=1, space="DRAM") as dramp:

            hT = [constp.tile([128, SHARD], F16, tag=f"hT{dh}", name=f"hT{dh}")
                  for dh in range(2)]
            h_nm = constp.tile([128, G, D], F16, tag="h_nm", name="h_nm")
            idx_sb = constp.tile([128, NW], I32, tag="idx", name="idx_sb")
            basis_sb = constp.tile([128, NB, 2, D], F16, tag="basis",
                                   name="basis_sb")
            root_sb = constp.tile([128, 2, D], F16, tag="root", name="root_sb")
            eps_sb = constp.tile([128, 1], F32, tag="eps", name="eps_sb")
            nc.vector.memset(eps_sb[:], LN_EPS)
            if has_bias:
                ones_sb = constp.tile([128, 128], F16, tag="ones", name="ones_sb")
                nc.vector.memset(ones_sb[:], 1.0)
                bias_sb = constp.tile([128, D], F16, tag="bias", name="bias_sb")
            if has_affine:
                aff_sb = constp.tile([128, 2, D], F32, tag="aff", name="aff_sb")

            h1_shard = dramp.tile([SHARD, D], F16, tag="h1s", name="h1_shard")
            h1_full = dramp.tile([NTOT, D], F16, tag="h1f", name="h1_full",
                                 addr_space="Shared")

            nc.sync.dma_start(out=idx_sb[:], in_=idx_in[:, :])

            for _rep in range(reps):
                nc.sync.dma_start(
                    out=h_nm[:],
                    in_=x_shard[:, :].rearrange("(g p) d -> p g d", p=128))
                for dh in range(2):
                    nc.sync.dma_start(out=hT[dh][:],
                                      in_=x_shard[:, dh * 128:(dh + 1) * 128],
                                      transpose=True)

                for l in range(L):
                    src_tab = x_full[:, :] if l == 0 else h1_full[:]
                    nc.sync.dma_start(out=basis_sb[:],
                                      in_=basis_in[l].rearrange(
                                          "b h p d -> p b h d"))
                    nc.sync.dma_start(out=root_sb[:],
                                      in_=root_in[l].rearrange("h p d -> p h d"))
                    if has_bias:
                        nc.sync.dma_start(out=bias_sb[:], in_=bias_in[l])
                    if has_affine:
                        nc.sync.dma_start(out=aff_sb[:],
                                          in_=aff_in[l].rearrange(
                                              "a p d -> p a d"))

                    for g in range(G):
                        c0 = g * CPG
                        xg = ginp.tile([128, CPG, D], F16, tag="xg", name="xg")
                        if l == 0:
                            nc.sync.dma_start(out=xg[:],
                                              in_=xg1_in[:, c0:c0 + CPG, :])
                        else:
                            for c in range(CPG):
                                nc.gpsimd.indirect_dma_start(
                                    out=xg[:, c, :], out_offset=None,
                                    in_=src_tab,
                                    in_offset=bass.IndirectOffsetOnAxis(
                                        ap=idx_sb[:, c0 + c:c0 + c + 1],
                                        axis=0))
                        m_sb = mstp.tile([128, CPG, NB * WIN], F16, tag="m",
                                         name="m_sb")
                        nc.sync.dma_start(out=m_sb[:], in_=m_in[l, :, c0:c0 + CPG, :])

                        # scatter matmuls -> S^T, evacuated to SBUF fp16
                        sT = [sTp.tile([128, NB * 128], F16, tag=f"sT{dh}",
                                       name=f"sT{dh}") for dh in range(2)]
                        for dh in range(2):
                            for sub in range(2):
                                scp = scps.tile([128, 512], F32, tag="scp",
                                                name="scp")
                                for cc in range(8):
                                    c = sub * 8 + cc
                                    nc.tensor.matmul(
                                        out=scp[:, cc * 64:(cc + 1) * 64],
                                        lhsT=xg[:, c, dh * 128:(dh + 1) * 128],
                                        rhs=m_sb[:, c, :],
                                        start=True, stop=True)
                                dst_ap = (sT[dh][:]
                                          .rearrange("p (b c s) -> p c b s",
                                                     b=NB, c=CPG, s=WIN)
                                          [:, sub * 8:(sub + 1) * 8, :, :])
                                if dh == 0:
                                    nc.vector.tensor_copy(out=dst_ap, in_=scp[:])
                                else:
                                    nc.scalar.copy(out=dst_ap, in_=scp[:])

                        # agg = sum_b S_b @ basis_b + h @ root (+ bias)
                        agg = aggps.tile([128, D], F32, tag="agg", name="agg")
                        first = True
                        for b in range(NB):
                            for dh in range(2):
                                nc.tensor.matmul(
                                    out=agg[:],
                                    lhsT=sT[dh][:, b * 128:(b + 1) * 128],
                                    rhs=basis_sb[:, b, dh, :],
                                    start=first, stop=False)
                                first = False
                        for dh in range(2):
                            last = (dh == 1) and not has_bias
                            nc.tensor.matmul(
                                out=agg[:],
                                lhsT=hT[dh][:, g * 128:(g + 1) * 128],
                                rhs=root_sb[:, dh, :],
                                start=False, stop=last)
                        if has_bias:
                            nc.tensor.matmul(
                                out=agg[:], lhsT=ones_sb[0:1, :],
                                rhs=bias_sb[0:1, :], start=False, stop=True)

                        # gelu (exact) + residual + layernorm
                        y = epp.tile([128, D], F32, tag="y", name="y")
                        ysum = epp.tile([128, 1], F32, tag="ysum", name="ysum")
                        gel = epp.tile([128, D], F32, tag="gel", name="gel")
                        nc.scalar.activation(out=gel[:], in_=agg[:], func=AF.Gelu)
                        nc.vector.scalar_tensor_tensor(
                            out=y[:], in0=gel[:], scalar=0.0, in1=h_nm[:, g, :],
                            op0=ALU.add, op1=ALU.add, accum_out=ysum[:])
                        mean = epp.tile([128, 1], F32, tag="mean", name="mean")
                        nc.vector.tensor_scalar_mul(mean[:], ysum[:], 1.0 / D)
                        vscr = epp.tile([128, D], F32, tag="vscr", name="vscr")
                        vsum = epp.tile([128, 1], F32, tag="vsum", name="vsum")
                        nc.vector.scalar_tensor_tensor(
                            out=vscr[:], in0=y[:], scalar=mean[:], in1=y[:],
                            op0=ALU.subtract, op1=ALU.mult, accum_out=vsum[:])
                        std = epp.tile([128, 1], F32, tag="std", name="std")
                        nc.scalar.activation(out=std[:], in_=vsum[:],
                                             func=AF.Sqrt, scale=1.0 / D,
                                             bias=eps_sb[:])
                        rstd = epp.tile([128, 1], F32, tag="rstd", name="rstd")
                        nc.vector.reciprocal(rstd[:], std[:])

                        dest = (h_nm[:, g, :] if l == 0 else None)
                        if has_affine:
                            hn = epp.tile([128, D], F32, tag="hn", name="hn")
                            nc.vector.tensor_scalar(
                                out=hn[:], in0=y[:], scalar1=mean[:],
                                scalar2=rstd[:], op0=ALU.subtract, op1=ALU.mult)
                            tgt = dest if l == 0 else None
                            if tgt is None:
                                tgt = epp.tile([128, D], F16, tag="ostage",
                                               name="ostage")
                            gm = epp.tile([128, D], F32, tag="gm", name="gm")
                            nc.vector.tensor_tensor(out=gm[:], in0=hn[:],
                                                    in1=aff_sb[:, 0, :],
                                                    op=ALU.mult)
                            nc.vector.tensor_tensor(out=tgt[:], in0=gm[:],
                                                    in1=aff_sb[:, 1, :],
                                                    op=ALU.add)
                            ostage = tgt
                        else:
                            if l == 0:
                                nc.vector.tensor_scalar(
                                    out=h_nm[:, g, :], in0=y[:], scalar1=mean[:],
                                    scalar2=rstd[:], op0=ALU.subtract,
                                    op1=ALU.mult)
                            else:
                                ostage = epp.tile([128, D], F16, tag="ostage",
                                                  name="ostage")
                                nc.vector.tensor_scalar(
                                    out=ostage[:], in0=y[:], scalar1=mean[:],
                                    scalar2=rstd[:], op0=ALU.subtract,
                                    op1=ALU.mult)

                        if l == 0:
                            nc.sync.dma_start(
                                out=h1_shard[:].rearrange(
                                    "(g p) d -> p g d", p=128)[:, g, :],
                                in_=h_nm[:, g, :])
                        else:
                            nc.sync.dma_start(
                                out=out_t[:, :].rearrange(
                                    "(g p) d -> p g d", p=128)[:, g, :],
                                in_=ostage[:])

                    if l == 0:
                        nc.gpsimd.collective_compute(
                            "AllGather", ALU.bypass, replica_groups=rg,
                            ins=[h1_shard.opt()], outs=[h1_full.opt()])
                        for dh in range(2):
                            nc.sync.dma_start(
                                out=hT[dh][:],
                                in_=h1_shard[:, dh * 128:(dh + 1) * 128],
                                transpose=True)

    nc.compile()
    return nc


def kernel(**inputs) -> np.ndarray:
    per_core, has_bias, has_affine = _host_prep(inputs)
    nc = build_nc(has_bias=has_bias, has_affine=has_affine)
    res = run_bass_kernel_spmd(nc, per_core, core_ids=list(range(NCORES)))
    return np.concatenate(
        [res.results[k]["out"].astype(np.float32) for k in range(NCORES)],
        axis=0)


# revision 4
# speedup vs baseline: 5.8155x; 5.8155x over previous
"""Trainium2 Bass kernel for nn_RelGraphEncoderTG (2-layer RGCN encoder).

Self-contained: takes the FULL inputs (as produced by the problem's
setup_inputs), shards across 8 NeuronCores internally, and returns the FULL
[65536, 256] float32 output.

Algorithm / sharding
--------------------
Nodes are split contiguously across the 8 cores (8192 per core); edges are
partitioned by destination-node owner.  Per core, edges are bucketed by
destination into windows of 8 consecutive nodes; each window owns one
128-edge-slot "chunk" (padded; pads carry zero weight).  The per-relation
mean-normalised, basis-decomposed message aggregation is refactored as

    agg[v] = sum_b S[v,b,:] @ basis[b]     with
    S[v,b,:] = sum_{e: dst(e)=v} w[e,b] * h[src(e)],
    w[e,b]   = comp[type(e),b] / cnt[dst(e), type(e)]

so the sparse scatter becomes dense PE matmuls: per chunk

    S^T[dhalf, (b,slot)] += Xg[128e, 128d].T @ M[128e, 64]

where Xg holds gathered source features (layer 1: pre-gathered on the host
from the input x; layer 2: per-chunk indirect DMA from the all-gathered h1)
and M is a host-built weighted one-hot (w[e,b] at column b*8+slot).  Per
group of 128 nodes, agg accumulates 16 basis matmuls + 2 root matmuls in
PSUM, followed by exact gelu (ACT), residual and LayerNorm (fused DVE ops).
h1 is all-gathered in fp16 between the layers.  All matmul operands are
fp16 with fp32 PSUM accumulation.
"""
import numpy as np

import concourse.bass as bass
import concourse.mybir as mybir
import concourse.tile as tile
import concourse.bacc as bacc
from concourse.bass_utils import run_bass_kernel_spmd

F16 = mybir.dt.float16
F32 = mybir.dt.float32
I32 = mybir.dt.int32
AF = mybir.ActivationFunctionType
ALU = mybir.AluOpType

NCORES = 8
NTOT = 65536
SHARD = NTOT // NCORES       # 8192 nodes per core
D = 256
NB = 8
R = 32
L = 2
WIN = 8                      # dst nodes per window (= per chunk)
CPG = 16                     # chunks per group (group = 128 dst nodes)
NW = SHARD // WIN            # windows == chunks per core per layer (1024)
G = SHARD // 128             # node groups per core (64)
LN_EPS = 1e-5


def _host_prep(inputs):
    x = np.asarray(inputs["x_flat"], np.float32)
    ei = np.asarray(inputs["edge_index"], np.int64)
    et = np.asarray(inputs["edge_type"], np.int64).astype(np.int32)
    basis = np.asarray(inputs["basis"], np.float32)
    comp = np.asarray(inputs["comp"], np.float32)
    root = np.asarray(inputs["root"], np.float32)
    bias = np.asarray(inputs["bias"], np.float32)
    gamma = np.asarray(inputs["ln_gamma"], np.float32)
    beta = np.asarray(inputs["ln_beta"], np.float32)
    mask = np.asarray(inputs["valid_mask_flat"])

    assert mask.all(), "kernel assumes valid_mask all-true (spec fill=ones)"
    has_bias = bool(np.any(bias != 0))
    has_affine = bool(np.any(gamma != 1) or np.any(beta != 0))

    src = ei[0].astype(np.int64)
    dst = ei[1].astype(np.int64)
    seg = dst * R + et
    cnt = np.bincount(seg, minlength=NTOT * R).astype(np.float32)
    norm = 1.0 / np.maximum(cnt[seg], 1.0)
    w = comp[:, et, :] * norm[None, :, None]        # [L, E, NB]

    x16 = x.astype(np.float16)

    per_core = []
    for k in range(NCORES):
        lo, hi = k * SHARD, (k + 1) * SHARD
        eidx = np.nonzero((dst >= lo) & (dst < hi))[0]
        dk = dst[eidx] - lo
        win = (dk >> 3).astype(np.int64)
        slot = (dk & 7).astype(np.int64)
        order = np.argsort(win, kind="stable")
        eidx, win, slot = eidx[order], win[order], slot[order]
        counts = np.bincount(win, minlength=NW)
        assert counts.max() <= 128, f"window degree {counts.max()} > 128"
        starts = np.concatenate([[0], np.cumsum(counts)[:-1]])
        pos = np.arange(len(eidx)) - starts[win]

        idx_t = np.zeros((128, NW), np.int32)       # pad -> row 0 (zero weight)
        idx_t[pos, win] = src[eidx].astype(np.int32)
        xg1 = np.zeros((128, NW, D), np.float16)
        xg1[pos, win, :] = x16[src[eidx]]
        m_tab = np.zeros((L, 128, NW, NB * WIN), np.float16)
        cols = (np.arange(NB)[None, :] * WIN) + slot[:, None]
        for l in range(L):
            m_tab[l, pos[:, None], win[:, None], cols] = \
                w[l, eidx, :].astype(np.float16)

        per_core.append({
            "x_full": x16,
            "x_shard": x16[lo:hi],
            "idx_t": idx_t,
            "xg1": xg1,
            "m_tab": m_tab,
            "basis_in": np.ascontiguousarray(
                basis.reshape(L, NB, 2, 128, D).astype(np.float16)),
            "root_in": np.ascontiguousarray(
                root.reshape(L, 2, 128, D).astype(np.float16)),
            "bias_in": np.ascontiguousarray(
                np.broadcast_to(bias[:, None, :], (L, 128, D)).astype(np.float16)),
            "ln_aff": np.ascontiguousarray(
                np.stack([np.broadcast_to(gamma[:, None, :], (L, 128, D)),
                          np.broadcast_to(beta[:, None, :], (L, 128, D))],
                         axis=1).astype(np.float32)),
        })
    return per_core, has_bias, has_affine


def build_nc(has_bias=False, has_affine=False, reps=1):
    nc = bacc.Bacc("TRN2", target_bir_lowering=False, debug=False,
                   enable_asserts=False, num_devices=NCORES)

    x_full = nc.dram_tensor("x_full", [NTOT, D], F16, kind="ExternalInput")
    x_shard = nc.dram_tensor("x_shard", [SHARD, D], F16, kind="ExternalInput")
    idx_in = nc.dram_tensor("idx_t", [128, NW], I32, kind="ExternalInput")
    xg1_in = nc.dram_tensor("xg1", [128, NW, D], F16, kind="ExternalInput")
    m_in = nc.dram_tensor("m_tab", [L, 128, NW, NB * WIN], F16,
                          kind="ExternalInput")
    basis_in = nc.dram_tensor("basis_in", [L, NB, 2, 128, D], F16,
                              kind="ExternalInput")
    root_in = nc.dram_tensor("root_in", [L, 2, 128, D], F16,
                             kind="ExternalInput")
    bias_in = nc.dram_tensor("bias_in", [L, 128, D], F16, kind="ExternalInput")
    aff_in = nc.dram_tensor("ln_aff", [L, 2, 128, D], F32, kind="ExternalInput")
    out_t = nc.dram_tensor("out", [SHARD, D], F16, kind="ExternalOutput")

    rg = [list(range(NCORES))]

    with tile.TileContext(nc) as tc:
        with tc.tile_pool(name="const", bufs=1) as constp, \
             tc.tile_pool(name="gin", bufs=3) as ginp, \
             tc.tile_pool(name="mst", bufs=3) as mstp, \
             tc.tile_pool(name="sT", bufs=2) as sTp, \
             tc.tile_pool(name="ep", bufs=2) as epp, \
             tc.tile_pool(name="sc_ps", bufs=6, space="PSUM") as scps, \
             tc.tile_pool(name="agg_ps", bufs=2, space="PSUM") as aggps, \
             tc.tile_pool(name="dram", bufs=1, space="DRAM") as dramp:

            hT = [constp.tile([128, SHARD], F16, tag=f"hT{dh}", name=f"hT{dh}")
                  for dh in range(2)]
            h_nm = constp.tile([128, G, D], F16, tag="h_nm", name="h_nm")
            idx_sb = constp.tile([128, NW], I32, tag="idx", name="idx_sb")
            basis_sb = constp.tile([128, NB, 2, D], F16, tag="basis",
                                   name="basis_sb")
            root_sb = constp.tile([128, 2, D], F16, tag="root", name="root_sb")
            eps_sb = constp.tile([128, 1], F32, tag="eps", name="eps_sb")
            nc.vector.memset(eps_sb[:], LN_EPS)
            if has_bias:
                ones_sb = constp.tile([128, 128], F16, tag="ones", name="ones_sb")
                nc.vector.memset(ones_sb[:], 1.0)
                bias_sb = constp.tile([128, D], F16, tag="bias", name="bias_sb")
            if has_affine:
                aff_sb = constp.tile([128, 2, D], F32, tag="aff", name="aff_sb")

            nc.sync.dma_start(out=idx_sb[:], in_=idx_in[:, :])

            for _rep in range(reps):
                sfx = "" if reps == 1 else f"_{_rep}"
                h1_shard = dramp.tile([SHARD, D], F16, tag=f"h1s{sfx}",
                                      name=f"h1_shard{sfx}")
                h1_full = dramp.tile([NTOT, D], F16, tag=f"h1f{sfx}",
                                     name=f"h1_full{sfx}",
                                     addr_space="Shared")
                nc.sync.dma_start(
                    out=h_nm[:],
                    in_=x_shard[:, :].rearrange("(g p) d -> p g d", p=128))
                for dh in range(2):
                    nc.sync.dma_start(out=hT[dh][:],
                                      in_=x_shard[:, dh * 128:(dh + 1) * 128],
                                      transpose=True)

                for l in range(L):
                    src_tab = x_full[:, :] if l == 0 else h1_full[:]
                    nc.sync.dma_start(out=basis_sb[:],
                                      in_=basis_in[l].rearrange(
                                          "b h p d -> p b h d"))
                    nc.sync.dma_start(out=root_sb[:],
                                      in_=root_in[l].rearrange("h p d -> p h d"))
                    if has_bias:
                        nc.sync.dma_start(out=bias_sb[:], in_=bias_in[l])
                    if has_affine:
                        nc.sync.dma_start(out=aff_sb[:],
                                          in_=aff_in[l].rearrange(
                                              "a p d -> p a d"))

                    for g in range(G):
                        c0 = g * CPG
                        xg = ginp.tile([128, CPG, D], F16, tag="xg", name="xg")
                        if l == 0:
                            nc.sync.dma_start(out=xg[:],
                                              in_=xg1_in[:, c0:c0 + CPG, :])
                        else:
                            for c in range(CPG):
                                nc.gpsimd.indirect_dma_start(
                                    out=xg[:, c, :], out_offset=None,
                                    in_=src_tab,
                                    in_offset=bass.IndirectOffsetOnAxis(
                                        ap=idx_sb[:, c0 + c:c0 + c + 1],
                                        axis=0))
                        m_sb = mstp.tile([128, CPG, NB * WIN], F16, tag="m",
                                         name="m_sb")
                        nc.sync.dma_start(out=m_sb[:], in_=m_in[l, :, c0:c0 + CPG, :])

                        # scatter matmuls -> S^T, evacuated to SBUF fp16
                        sT = [sTp.tile([128, NB * 128], F16, tag=f"sT{dh}",
                                       name=f"sT{dh}") for dh in range(2)]
                        for dh in range(2):
                            for sub in range(2):
                                scp = scps.tile([128, 512], F32, tag="scp",
                                                name="scp")
                                for cc in range(8):
                                    c = sub * 8 + cc
                                    nc.tensor.matmul(
                                        out=scp[:, cc * 64:(cc + 1) * 64],
                                        lhsT=xg[:, c, dh * 128:(dh + 1) * 128],
                                        rhs=m_sb[:, c, :],
                                        start=True, stop=True)
                                dst_ap = (sT[dh][:]
                                          .rearrange("p (b c s) -> p c b s",
                                                     b=NB, c=CPG, s=WIN)
                                          [:, sub * 8:(sub + 1) * 8, :, :])
                                nc.vector.tensor_copy(out=dst_ap, in_=scp[:])

                        # agg = sum_b S_b @ basis_b + h @ root (+ bias)
                        agg = aggps.tile([128, D], F32, tag="agg", name="agg")
                        first = True
                        for b in range(NB):
                            for dh in range(2):
                                nc.tensor.matmul(
                                    out=agg[:],
                                    lhsT=sT[dh][:, b * 128:(b + 1) * 128],
                                    rhs=basis_sb[:, b, dh, :],
                                    start=first, stop=False)
                                first = False
                        for dh in range(2):
                            last = (dh == 1) and not has_bias
                            nc.tensor.matmul(
                                out=agg[:],
                                lhsT=hT[dh][:, g * 128:(g + 1) * 128],
                                rhs=root_sb[:, dh, :],
                                start=False, stop=last)
                        if has_bias:
                            nc.tensor.matmul(
                                out=agg[:], lhsT=ones_sb[0:1, :],
                                rhs=bias_sb[0:1, :], start=False, stop=True)

                        # gelu (exact) + residual + layernorm
                        y = epp.tile([128, D], F32, tag="y", name="y")
                        ysum = epp.tile([128, 1], F32, tag="ysum", name="ysum")
                        gel = epp.tile([128, D], F32, tag="gel", name="gel")
                        nc.scalar.activation(out=gel[:], in_=agg[:], func=AF.Gelu)
                        nc.vector.scalar_tensor_tensor(
                            out=y[:], in0=gel[:], scalar=0.0, in1=h_nm[:, g, :],
                            op0=ALU.add, op1=ALU.add, accum_out=ysum[:])
                        mean = epp.tile([128, 1], F32, tag="mean", name="mean")
                        nc.vector.tensor_scalar_mul(mean[:], ysum[:], 1.0 / D)
                        vscr = epp.tile([128, D], F32, tag="vscr", name="vscr")
                        vsum = epp.tile([128, 1], F32, tag="vsum", name="vsum")
                        nc.vector.scalar_tensor_tensor(
                            out=vscr[:], in0=y[:], scalar=mean[:], in1=y[:],
                            op0=ALU.subtract, op1=ALU.mult, accum_out=vsum[:])
                        std = epp.tile([128, 1], F32, tag="std", name="std")
                        nc.scalar.activation(out=std[:], in_=vsum[:],
                                             func=AF.Sqrt, scale=1.0 / D,
                                             bias=eps_sb[:])
                        rstd = epp.tile([128, 1], F32, tag="rstd", name="rstd")
                        nc.vector.reciprocal(rstd[:], std[:])

                        dest = (h_nm[:, g, :] if l == 0 else None)
                        if has_affine:
                            hn = epp.tile([128, D], F32, tag="hn", name="hn")
                            nc.vector.tensor_scalar(
                                out=hn[:], in0=y[:], scalar1=mean[:],
                                scalar2=rstd[:], op0=ALU.subtract, op1=ALU.mult)
                            tgt = dest if l == 0 else None
                            if tgt is None:
                                tgt = epp.tile([128, D], F16, tag="ostage",
                                               name="ostage")
                            gm = epp.tile([128, D], F32, tag="gm", name="gm")
                            nc.vector.tensor_tensor(out=gm[:], in0=hn[:],
                                                    in1=aff_sb[:, 0, :],
                                                    op=ALU.mult)
                            nc.vector.tensor_tensor(out=tgt[:], in0=gm[:],
                                                    in1=aff_sb[:, 1, :],
                                                    op=ALU.add)
                            ostage = tgt
                        else:
                            if l == 0:
                                nc.vector.tensor_scalar(
                                    out=h_nm[:, g, :], in0=y[:], scalar1=mean[:],
                                    scalar2=rstd[:], op0=ALU.subtract,
                                    op1=ALU.mult)
                            else:
                                ostage = epp.tile([128, D], F16, tag="ostage",
                                                  name="ostage")
                                nc.vector.tensor_scalar(
                                    out=ostage[:], in0=y[:], scalar1=mean[:],
                                    scalar2=rstd[:], op0=ALU.subtract,
                                    op1=ALU.mult)

                        if l == 0:
                            nc.sync.dma_start(
                                out=h1_shard[:].rearrange(
                                    "(g p) d -> p g d", p=128)[:, g, :],
                                in_=h_nm[:, g, :])
                        else:
                            nc.sync.dma_start(
                                out=out_t[:, :].rearrange(
                                    "(g p) d -> p g d", p=128)[:, g, :],
                                in_=ostage[:])

                    if l == 0:
                        nc.gpsimd.collective_compute(
                            "AllGather", ALU.bypass, replica_groups=rg,
                            ins=[h1_shard.opt()], outs=[h1_full.opt()])
                        for dh in range(2):
                            nc.sync.dma_start(
                                out=hT[dh][:],
                                in_=h1_shard[:, dh * 128:(dh + 1) * 128],
                                transpose=True)

    nc.compile()
    return nc


def kernel(**inputs) -> np.ndarray:
    per_core, has_bias, has_affine = _host_prep(inputs)
    nc = build_nc(has_bias=has_bias, has_affine=has_affine)
    res = run_bass_kernel_spmd(nc, per_core, core_ids=list(range(NCORES)))
    return np.concatenate(
        [res.results[k]["out"].astype(np.float32) for k in range(NCORES)],
        axis=0)
